# revision 15
# baseline (speedup 1.0000x reference)
"""GNN message-passing (NBFNet-style) Trainium2 kernel: host prep + Bass/Tile builder.

v2 all-SBUF design (per core, 2 batches packed as 128 = 2b x 64d partitions):
  - node-state tensors [128, NPAD] bf16 live entirely in SBUF, in per-snapshot
    "rank space" (nodes sorted by in-degree desc)
  - edge message gather via GPSIMD indirect_copy (on-chip column gather) from
    the SBUF layer-input table; no HBM roundtrip, no transposes
  - per-edge relation embeddings precomputed on host, streamed as sequential
    DMA ([128, E] bf16 per (snap, layer))
  - segment stats via degree-sorted rounds as in-place tensor_tensor prefix
    updates; sum and sq-sum share one paired [128, 2, NPAD] accumulator so one
    TT updates both; min-chain tail rounds run on the GPSIMD/Pool engine to
    offload DVE
  - PNA combine: 3 PSUM banks (per scale k), 13 matmuls + identity matmul to
    add the scale-combined k=1/k=2 terms back into bank0, ACT relu evacuation
"""
import sys
import contextlib

import numpy as np

sys.path.insert(0, "/opt/trn_rl_repo")
import ml_dtypes  # noqa: E402
import concourse.bass as bass  # noqa: E402
import concourse.tile as tile  # noqa: E402
from concourse import bacc, mybir, bass_utils  # noqa: E402

BF16 = mybir.dt.bfloat16
F32 = mybir.dt.float32
U16 = mybir.dt.uint16
AF = mybir.ActivationFunctionType
OP = mybir.AluOpType
nbf = ml_dtypes.bfloat16

N = 5000
NPAD = 5120
E = 30000
D = 64
B = 16
NCORES = 8
BL = B // NCORES  # 2
HIST = 2
NL = 2
NEG = 32
NREL2 = 400
EPS = 1e-6
CH = 5120        # edge msg chunk (cols)
NTILE = 512      # matmul node tile
BLK = 2560       # stats post-processing block
EPS_CLIP_SCALE = 1e-2
SQ_POOL_ROUND = 2    # sq-chain rounds >= this run on Pool engine (add only)
GMAX = 1024          # indirect_copy max dst cols per instruction (ISA limit)

STATS = ("mean", "max", "min", "std")


def _wrap16(idx):
    """[L] int -> [128, L/16] uint16 wrapped in 16 partitions, replicated x8."""
    L = len(idx)
    assert L % 16 == 0
    w = np.asarray(idx, np.int64).reshape(L // 16, 16).T.astype(np.uint16)
    return np.tile(w, (8, 1))


def prep_snap(src, dst, et):
    """Host index preprocessing for one snapshot."""
    src = np.asarray(src, np.int64)
    dst = np.asarray(dst, np.int64)
    et = np.asarray(et, np.int64)
    indeg = np.bincount(dst, minlength=N).astype(np.int64)
    order = np.argsort(-indeg, kind="stable")  # rank -> node
    rank_of = np.empty(N, np.int64)
    rank_of[order] = np.arange(N)

    er = rank_of[dst]
    eord = np.argsort(er, kind="stable")  # edges sorted by dst rank
    er_s = er[eord]
    starts = np.searchsorted(er_s, er_s, side="left")
    slot = np.arange(E) - starts  # slot within dst group (0-indexed round)
    Rmax = int(indeg.max())
    W = np.array([int(np.count_nonzero(indeg > r)) for r in range(Rmax)], np.int64)
    off = np.concatenate([[0], np.cumsum(W)])
    assert off[-1] == E
    pos = off[slot] + er_s
    assert len(np.unique(pos)) == E
    src_rm = np.zeros(E, np.int64)
    et_rm = np.zeros(E, np.int64)
    src_rm[pos] = src[eord]
    et_rm[pos] = et[eord]

    # chunk op lists: per chunk (c0, c1, ops), ops = (msg_off, acc_off, width, round)
    chunks = []
    for c0 in range(0, E, CH):
        c1 = min(c0 + CH, E)
        ops = []
        for r in range(Rmax):
            g0, g1 = int(off[r]), int(off[r] + W[r])
            a, b_ = max(g0, c0), min(g1, c1)
            if a < b_:
                ops.append((a - c0, a - g0, b_ - a, r))
        chunks.append((c0, c1, ops))

    deg = (indeg + 1).astype(np.float64)
    scl = np.log(deg)
    scl = scl / scl.mean()
    iscl = 1.0 / np.clip(scl, EPS_CLIP_SCALE, None)
    invdeg = 1.0 / deg
    mask = (deg > 1).astype(np.float64)

    def pad_rank(x, fill):
        out = np.full(NPAD, fill, np.float64)
        out[:N] = x[order]
        return out

    v = np.stack([pad_rank(invdeg, 1.0), pad_rank(scl, 1.0), pad_rank(iscl, 1.0),
                  pad_rank(mask, 1.0)])
    return dict(
        indeg=indeg, order=order, rank_of=rank_of, W0=int(W[0]),
        src_rm=src_rm, et_rm=et_rm, chunks=chunks, v=v.astype(nbf),
    )


def preprocess(inputs):
    qt = np.asarray(inputs["query_triple"], np.int64)  # [B, NEG, 3]
    h_index, r_index, t_index = qt[..., 0], qt[..., 1], qt[..., 2]
    is_t_neg = np.all(h_index == h_index[:, :1], axis=-1, keepdims=True)
    h_i = np.where(is_t_neg, h_index, t_index)
    t_i = np.where(is_t_neg, t_index, h_index)
    r_i = np.where(is_t_neg, r_index, r_index + NREL2 // 2)

    ei = np.asarray(inputs["edge_index"], np.int64)
    etp = np.asarray(inputs["edge_type"], np.int64)
    snaps = [prep_snap(ei[s, 0], ei[s, 1], etp[s]) for s in range(HIST)]
    for s in range(HIST):
        sn = snaps[s]
        sn["xidx_w"] = _wrap16(sn["rank_of"][sn["src_rm"]])
    # snap1 init gather: rank1 col j <- rank0 col r0[order1[j]]
    perm = np.zeros(NPAD, np.int64)
    perm[:N] = snaps[0]["rank_of"][snaps[1]["order"]]
    perm1_w = _wrap16(perm)

    qw = np.asarray(inputs["query_weight"], np.float32)      # [NREL2, D]
    rel = np.asarray(inputs["rel_embs"], np.float32)         # [NL, NREL2, D]
    lw = np.asarray(inputs["layer_Ws"], np.float32)          # [NL, 13*D, D]
    lb = np.asarray(inputs["layer_bs"], np.float32)          # [NL, D]
    w1 = np.asarray(inputs["mlp_w1"], np.float32)            # [128, 128]
    b1 = np.asarray(inputs["mlp_b1"], np.float32)            # [128]
    w2 = np.asarray(inputs["mlp_w2"], np.float32)            # [128, 1]
    b2 = np.asarray(inputs["mlp_b2"], np.float32)            # [1]

    # weight chunk tables: chunk 0 = rows 0:64 (x); chunk 1+si*3+k = rows
    # 64 + d*12 + si*3 + k (agg layout: ((d*4+si)*3+k))
    Wc = np.zeros((NL, 13, D, D), np.float32)
    for li in range(NL):
        Wc[li, 0] = lw[li, :D]
        for si in range(4):
            for k in range(3):
                rows = 64 + np.arange(D) * 12 + si * 3 + k
                Wc[li, 1 + si * 3 + k] = lw[li, rows]

    # per-edge rel tables [HIST*NL, 128, E] bf16 (rows = both batches x 64d)
    relw = np.zeros((HIST * NL, 128, E), nbf)
    for s in range(HIST):
        ets = snaps[s]["et_rm"]
        for li in range(NL):
            t = rel[li][ets].T.astype(nbf)   # [D, E]
            relw[s * NL + li, :D] = t
            relw[s * NL + li, D:] = t

    ident = np.eye(128, dtype=np.float32).astype(nbf)

    per_core = []
    for c in range(NCORES):
        bsl = slice(c * BL, (c + 1) * BL)
        q = qw[r_i[bsl, 0]]                      # [BL, D] f32
        init0 = np.zeros((128, NPAD), np.float32)
        for b in range(BL):
            init0[b * D:(b + 1) * D, snaps[0]["rank_of"][h_i[c * BL + b, 0]]] = q[b]
        tid = np.zeros(128, np.int64)
        tid[:BL * NEG] = snaps[1]["rank_of"][t_i[bsl].reshape(-1)]
        per_core.append(dict(
            init0=init0.astype(nbf), qvec=q.astype(np.float32),
            tidx_w=_wrap16(tid),
        ))

    return dict(
        snaps=snaps, Wc=Wc, relw=relw, lbias=lb, perm1_w=perm1_w, ident=ident,
        w1=w1, b1=b1, w2=w2, b2=b2,
        per_core=per_core, h_i=h_i, t_i=t_i, r_i=r_i,
    )


def build(cfg, debug=()):
    nc = bacc.Bacc("TRN2", target_bir_lowering=False, debug=False)
    snaps = cfg["snaps"]

    # ---- DRAM tensors
    d_init0 = nc.dram_tensor("init0", [128, NPAD], BF16, kind="ExternalInput")
    d_qvec = nc.dram_tensor("qvec", [BL, D], F32, kind="ExternalInput")
    d_tidx = nc.dram_tensor("tidx", [128, 8], U16, kind="ExternalInput")
    d_xidx = nc.dram_tensor("xidx", [HIST, 128, E // 16], U16, kind="ExternalInput")
    d_perm1 = nc.dram_tensor("perm1", [128, NPAD // 16], U16, kind="ExternalInput")
    d_relw = nc.dram_tensor("relw", [HIST * NL, 128, E], BF16, kind="ExternalInput")
    d_v = nc.dram_tensor("vvec", [HIST, 4, NPAD], BF16, kind="ExternalInput")
    d_wc = nc.dram_tensor("wchunks", [NL, 13, D, D], F32, kind="ExternalInput")
    d_lb = nc.dram_tensor("lbias", [NL, D], F32, kind="ExternalInput")
    d_ident = nc.dram_tensor("ident", [128, 128], BF16, kind="ExternalInput")
    d_w1 = nc.dram_tensor("w1", [128, 128], F32, kind="ExternalInput")
    d_b1 = nc.dram_tensor("b1", [128], F32, kind="ExternalInput")
    d_w2 = nc.dram_tensor("w2", [128, 1], F32, kind="ExternalInput")
    d_b2 = nc.dram_tensor("b2", [1], F32, kind="ExternalInput")
    d_scores = nc.dram_tensor("scores", [BL, NEG], F32, kind="ExternalOutput")

    dbg_tensors = {}

    def dbg(name, shape, dtype):
        if name in debug:
            dbg_tensors[name] = nc.dram_tensor("dbg_" + name, shape, dtype,
                                               kind="ExternalOutput")
            return dbg_tensors[name]
        return None

    with tile.TileContext(nc) as tc, contextlib.ExitStack() as ctx:
        p_idx = ctx.enter_context(tc.tile_pool(name="idx", bufs=1))
        p_const = ctx.enter_context(tc.tile_pool(name="const", bufs=1))
        p_v = ctx.enter_context(tc.tile_pool(name="vrep", bufs=1))
        p_init = ctx.enter_context(tc.tile_pool(name="init", bufs=2))
        p_edge = ctx.enter_context(tc.tile_pool(name="edge", bufs=2))
        p_rel = ctx.enter_context(tc.tile_pool(name="rel", bufs=2))
        p_acc = ctx.enter_context(tc.tile_pool(name="acc", bufs=1))
        p_x = ctx.enter_context(tc.tile_pool(name="x", bufs=2))
        p_t12 = ctx.enter_context(tc.tile_pool(name="t12", bufs=4))
        p_ps = ctx.enter_context(tc.tile_pool(name="ps", bufs=6, space="PSUM"))
        p_ps2 = ctx.enter_context(tc.tile_pool(name="ps2", bufs=1, space="PSUM"))
        p_misc = ctx.enter_context(tc.tile_pool(name="misc", bufs=1))

        # ---- setup: weights
        wmm = {}
        for li in range(NL):
            for c in range(13):
                wstg = p_misc.tile([D, D], F32, name="wstg", tag="wstg", bufs=2)
                nc.sync.dma_start(wstg[:], bass.AP(
                    tensor=d_wc, offset=(li * 13 + c) * D * D,
                    ap=[[D, D], [1, D]]))
                t = p_const.tile([128, 128], BF16, name=f"wmm{li}_{c}",
                                 tag=f"wmm{li}_{c}")
                nc.vector.memset(t[:], 0.0)
                nc.vector.tensor_copy(t[0:D, 0:D], wstg[:])
                nc.vector.tensor_copy(t[D:128, D:128], wstg[:])
                wmm[(li, c)] = t
        identsb = p_const.tile([128, 128], BF16, name="identsb", tag="identsb")
        nc.sync.dma_start(identsb[:], d_ident.ap()[:])
        w1stage = p_misc.tile([128, 128], F32, name="w1stage", tag="w1stage")
        nc.sync.dma_start(w1stage[:], d_w1.ap()[:])
        w1sb = p_const.tile([128, 128], BF16, name="w1sb", tag="w1sb")
        nc.vector.tensor_copy(w1sb[:], w1stage[:])
        w2stage = p_misc.tile([128, 1], F32, name="w2stage", tag="w2stage")
        nc.sync.dma_start(w2stage[:], d_w2.ap()[:])
        w2sb = p_const.tile([128, 1], BF16, name="w2sb", tag="w2sb")
        nc.vector.tensor_copy(w2sb[:], w2stage[:])
        b1sb = p_const.tile([128, 1], F32, name="b1sb", tag="b1sb")
        nc.sync.dma_start(b1sb[:], bass.AP(tensor=d_b1, offset=0, ap=[[1, 128], [1, 1]]))
        b2sb = p_const.tile([1, 1], F32, name="b2sb", tag="b2sb")
        nc.sync.dma_start(b2sb[:], bass.AP(tensor=d_b2, offset=0, ap=[[1, 1], [1, 1]]))
        lbsb = p_const.tile([128, NL], F32, name="lbsb", tag="lbsb")
        for li in range(NL):
            nc.sync.dma_start(lbsb[:, li:li + 1], bass.AP(
                tensor=d_lb, offset=li * D, ap=[[0, 2], [1, D]]))
        qsb = []
        for b in range(BL):
            t = p_const.tile([128, 1], F32, name=f"qsb{b}", tag=f"qsb{b}")
            nc.sync.dma_start(t[D:128, :], bass.AP(
                tensor=d_qvec, offset=b * D, ap=[[1, D], [1, 1]]))
            qsb.append(t)
        epssb = p_const.tile([128, 1], F32, name="epssb", tag="epssb")
        nc.vector.memset(epssb[:], EPS)
        tidx_sb = p_const.tile([128, 8], U16, name="tidx", tag="tidx")
        nc.sync.dma_start(tidx_sb[:], d_tidx.ap()[:])

        x_prev = None
        init2_prev = None
        for s in range(HIST):
            sn = snaps[s]
            # ---- per-snap index arrays + v tensors
            xidx_sb = p_idx.tile([128, E // 16], U16, name="xidx", tag="xidx")
            nc.sync.dma_start(xidx_sb[:], d_xidx.ap()[s])
            vrep = []
            for j in range(4):
                t = p_v.tile([128, NPAD], BF16, name=f"v{j}", tag=f"v{j}")
                nc.sync.dma_start(t[:], bass.AP(
                    tensor=d_v, offset=(s * 4 + j) * NPAD, ap=[[0, 128], [1, NPAD]]))
                vrep.append(t)
            vinv, vs1, vs2, vmask = vrep

            # ---- initial (paired with its square) in rank-s space
            init2 = p_init.tile([128, 2, NPAD], BF16, name="init2", tag="init2")
            if s == 0:
                nc.sync.dma_start(init2[:, 0, :], d_init0.ap()[:])
            else:
                perm_sb = p_idx.tile([128, NPAD // 16], U16, name="perm", tag="perm")
                nc.sync.dma_start(perm_sb[:], d_perm1.ap()[:])
                tmp = p_rel.tile([128, CH], BF16, name="blend", tag="relg")
                tmp = tmp[:, :NPAD]
                for g0 in range(0, NPAD, GMAX):
                    g1 = g0 + GMAX
                    isl = slice(g0 // 16, g1 // 16)
                    nc.gpsimd.indirect_copy(init2[:, 0, g0:g1],
                                            init2_prev[:, 0, :],
                                            perm_sb[:, isl], True)
                    nc.gpsimd.indirect_copy(tmp[:, g0:g1], x_prev[:],
                                            perm_sb[:, isl], True)
                nc.vector.tensor_tensor(out=init2[:, 0, :], in0=init2[:, 0, :],
                                        in1=tmp, op=OP.add)
                nc.vector.tensor_scalar(out=init2[:, 0, :], in0=init2[:, 0, :],
                                        scalar1=0.5, scalar2=None, op0=OP.mult)
            nc.scalar.activation(init2[:, 1, :], init2[:, 0, :], AF.Square)
            init2_prev = init2
            if (t_ := dbg(f"initial{s}", [128, 2, NPAD], BF16)) is not None:
                nc.sync.dma_start(t_.ap()[:], init2[:])

            for li in range(NL):
                x_in = init2[:, 0, :] if li == 0 else x_prev[:]
                W0 = sn["W0"]
                # ---- stats accumulators
                acc2 = p_acc.tile([128, 2, NPAD], BF16, name="acc2", tag="acc2")
                accmax = p_acc.tile([128, NPAD], BF16, name="accmax", tag="accmax")
                accmin = p_acc.tile([128, NPAD], BF16, name="accmin", tag="accmin")
                # suffix init (nodes with indeg==0 never touched by rounds)
                if W0 < NPAD:
                    nc.scalar.copy(acc2[:, :, W0:], init2[:, :, W0:])
                    nc.scalar.copy(accmax[:, W0:], init2[:, 0, W0:])
                    nc.scalar.copy(accmin[:, W0:], init2[:, 0, W0:])

                # ---- edge chunks: gathers first (Pool), then DVE/Act/round ops
                nch = len(sn["chunks"])

                def emit_gather(ci):
                    c0, c1, _ = sn["chunks"][ci]
                    xg = p_edge.tile([128, CH], BF16, name="xg", tag="xg")
                    for g0 in range(c0, c1, GMAX):
                        g1 = min(g0 + GMAX, c1)
                        nc.gpsimd.indirect_copy(
                            xg[:, g0 - c0:g1 - c0], x_in,
                            xidx_sb[:, g0 // 16:g1 // 16], True)
                    return xg

                xgs = {0: emit_gather(0)}
                for ci, (c0, c1, ops) in enumerate(sn["chunks"]):
                    w = c1 - c0
                    xg = xgs.pop(ci)
                    relg = p_rel.tile([128, CH], BF16, name="relg", tag="relg")
                    nc.sync.dma_start(relg[:, :w],
                                      d_relw.ap()[s * NL + li][:, c0:c1])
                    msg = xg[:, :w]
                    nc.vector.tensor_tensor(out=msg, in0=msg, in1=relg[:, :w],
                                            op=OP.mult)
                    # msgsq overwrites the (dead) rel values in-place
                    nc.scalar.activation(relg[:, :w], msg, AF.Square)
                    if ci == 0 and (t_ := dbg(f"xg{s}{li}", [128, CH], BF16)) is not None:
                        nc.sync.dma_start(t_.ap()[:, :w], msg)
                    if ci + 1 < nch:
                        xgs[ci + 1] = emit_gather(ci + 1)
                    pool_sq = []
                    for (mo, ao, wd, r) in ops:
                        # sum
                        in0 = init2[:, 0, ao:ao + wd] if r == 0 else acc2[:, 0, ao:ao + wd]
                        nc.vector.tensor_tensor(
                            out=acc2[:, 0, ao:ao + wd], in0=in0,
                            in1=xg[:, mo:mo + wd], op=OP.add)
                        # sq-sum: tail rounds offloaded to Pool
                        in0 = init2[:, 1, ao:ao + wd] if r == 0 else acc2[:, 1, ao:ao + wd]
                        args = dict(out=acc2[:, 1, ao:ao + wd], in0=in0,
                                    in1=relg[:, mo:mo + wd], op=OP.add)
                        if r >= SQ_POOL_ROUND:
                            pool_sq.append(args)
                        else:
                            nc.vector.tensor_tensor(**args)
                        # max
                        in0 = init2[:, 0, ao:ao + wd] if r == 0 else accmax[:, ao:ao + wd]
                        nc.vector.tensor_tensor(
                            out=accmax[:, ao:ao + wd], in0=in0,
                            in1=xg[:, mo:mo + wd], op=OP.max)
                        # min
                        in0 = init2[:, 0, ao:ao + wd] if r == 0 else accmin[:, ao:ao + wd]
                        nc.vector.tensor_tensor(
                            out=accmin[:, ao:ao + wd], in0=in0,
                            in1=xg[:, mo:mo + wd], op=OP.min)
                    for args in pool_sq:
                        nc.gpsimd.tensor_tensor(**args)

                # ---- stats post, per block (high block first to unlock matmuls)
                for b in range(NPAD // BLK - 1, -1, -1):
                    bsl = slice(b * BLK, (b + 1) * BLK)
                    nc.vector.tensor_tensor(
                        out=acc2[:, :, bsl], in0=acc2[:, :, bsl],
                        in1=vinv[:, bsl].unsqueeze(1).broadcast_to([128, 2, BLK]),
                        op=OP.mult)
                    msq = p_rel.tile([128, CH], BF16, name="msq", tag="relg")
                    msq = msq[:, :BLK]
                    nc.scalar.activation(msq, acc2[:, 0, bsl], AF.Square)
                    nc.vector.tensor_tensor(out=acc2[:, 1, bsl],
                                            in0=acc2[:, 1, bsl], in1=msq,
                                            op=OP.subtract)
                    nc.scalar.activation(acc2[:, 1, bsl], acc2[:, 1, bsl], AF.Relu)
                    nc.vector.tensor_tensor(out=acc2[:, 1, bsl],
                                            in0=acc2[:, 1, bsl],
                                            in1=vmask[:, bsl], op=OP.mult)
                    nc.scalar.activation(acc2[:, 1, bsl], acc2[:, 1, bsl],
                                         AF.Sqrt, bias=epssb[:, 0:1])
                if (t_ := dbg(f"stats{s}{li}", [128, 2, NPAD], BF16)) is not None:
                    nc.sync.dma_start(t_.ap()[:], acc2[:])
                if (t_ := dbg(f"statmm{s}{li}", [128, 2, NPAD], BF16)) is not None:
                    nc.sync.dma_start(t_.ap()[:, 0, :], accmax[:])
                    nc.sync.dma_start(t_.ap()[:, 1, :], accmin[:])

                # ---- matmuls: high tiles first
                xnext = p_x.tile([128, NPAD], BF16, name="xnext", tag="xnext")
                stat_rhs = [acc2[:, 0, :], accmax[:], accmin[:], acc2[:, 1, :]]
                for t in range(NPAD // NTILE - 1, -1, -1):
                    tsl = slice(t * NTILE, (t + 1) * NTILE)
                    ps = [p_ps.tile([128, NTILE], F32, name=f"ps{k}", tag="ps")
                          for k in range(3)]
                    for k in (1, 2):
                        for si in range(4):
                            nc.tensor.matmul(
                                out=ps[k][:], lhsT=wmm[(li, 1 + si * 3 + k)][:],
                                rhs=stat_rhs[si][:, tsl],
                                start=(si == 0), stop=(si == 3))
                    nc.tensor.matmul(out=ps[0][:], lhsT=wmm[(li, 0)][:],
                                     rhs=x_in[:, tsl], start=True, stop=False)
                    for si in range(4):
                        nc.tensor.matmul(
                            out=ps[0][:], lhsT=wmm[(li, 1 + si * 3)][:],
                            rhs=stat_rhs[si][:, tsl], start=False, stop=False)
                    t1 = p_t12.tile([128, NTILE], BF16, name="t1", tag="t1")
                    t2 = p_t12.tile([128, NTILE], BF16, name="t2", tag="t2")
                    nc.scalar.copy(t1[:], ps[1][:])
                    nc.scalar.copy(t2[:], ps[2][:])
                    nc.vector.tensor_tensor(out=t1[:], in0=t1[:],
                                            in1=vs1[:, tsl], op=OP.mult)
                    nc.vector.tensor_tensor(out=t2[:], in0=t2[:],
                                            in1=vs2[:, tsl], op=OP.mult)
                    nc.vector.tensor_tensor(out=t1[:], in0=t1[:], in1=t2[:],
                                            op=OP.add)
                    nc.tensor.matmul(out=ps[0][:], lhsT=identsb[:], rhs=t1[:],
                                     start=False, stop=True)
                    nc.scalar.activation(xnext[:, tsl], ps[0][:], AF.Relu,
                                         bias=lbsb[:, li:li + 1])
                if (t_ := dbg(f"x_s{s}_l{li}", [128, NPAD], BF16)) is not None:
                    nc.sync.dma_start(t_.ap()[:], xnext[:])
                x_prev = xnext

        # ---- final readout from x_prev (rank-1 space)
        tg = p_misc.tile([128, 128], BF16, name="tg", tag="tg")
        nc.gpsimd.indirect_copy(tg[:], x_prev[:], tidx_sb[:], True)
        for b in range(BL):
            ft = p_misc.tile([128, NEG], BF16, name=f"ft{b}", tag=f"ft{b}")
            if b == 0:
                nc.vector.tensor_copy(ft[0:D, :], tg[0:D, 0:NEG])
            else:
                nc.sync.dma_start(ft[0:D, :], tg[D:128, b * NEG:(b + 1) * NEG])
            nc.vector.memset(ft[D:128, :], 0.0)
            nc.vector.tensor_scalar(out=ft[D:128, :], in0=ft[D:128, :],
                                    scalar1=qsb[b][D:128, 0:1], scalar2=None,
                                    op0=OP.add)
            ps1 = p_ps2.tile([128, NEG], F32, name="mlp1", tag="mlp1")
            nc.tensor.matmul(out=ps1[:], lhsT=w1sb[:], rhs=ft[:], start=True, stop=True)
            hd = p_misc.tile([128, NEG], BF16, name=f"hd{b}", tag=f"hd{b}")
            nc.scalar.activation(hd[:], ps1[:], AF.Relu, bias=b1sb[:, 0:1])
            ps2 = p_ps2.tile([1, NEG], F32, name="mlp2", tag="mlp2")
            nc.tensor.matmul(out=ps2[:], lhsT=w2sb[:], rhs=hd[:], start=True, stop=True)
            sc = p_misc.tile([1, NEG], F32, name=f"sc{b}", tag=f"sc{b}")
            nc.vector.tensor_scalar(out=sc[:], in0=ps2[:], scalar1=b2sb[0:1, 0:1],
                                    scalar2=None, op0=OP.add)
            nc.sync.dma_start(d_scores.ap()[b:b + 1, :], sc[:])

    nc.compile()
    return nc, dbg_tensors


def make_in_maps(cfg):
    shared = {}
    shared["xidx"] = np.stack([cfg["snaps"][s]["xidx_w"] for s in range(HIST)])
    shared["perm1"] = cfg["perm1_w"]
    shared["relw"] = cfg["relw"]
    shared["vvec"] = np.stack([cfg["snaps"][s]["v"] for s in range(HIST)])
    shared["wchunks"] = cfg["Wc"]
    shared["lbias"] = cfg["lbias"]
    shared["ident"] = cfg["ident"]
    shared["w1"] = cfg["w1"]
    shared["b1"] = cfg["b1"]
    shared["w2"] = cfg["w2"]
    shared["b2"] = cfg["b2"]
    in_maps = []
    for c in range(NCORES):
        pc = cfg["per_core"][c]
        in_maps.append(dict(shared, init0=pc["init0"], qvec=pc["qvec"],
                            tidx=pc["tidx_w"]))
    return in_maps


_CACHE = {}


def _get_program(cfg, key):
    if key not in _CACHE:
        _CACHE[key] = build(cfg)
    return _CACHE[key]


def kernel(**inputs):
    cfg = preprocess(inputs)
    key = (np.asarray(inputs["edge_index"]).tobytes(),
           np.asarray(inputs["edge_type"]).tobytes())
    nc, _ = _get_program(cfg, key)
    in_maps = make_in_maps(cfg)
    res = bass_utils.run_bass_kernel_spmd(nc, in_maps, core_ids=list(range(NCORES)))
    scores = np.concatenate([res.results[c]["scores"] for c in range(NCORES)], axis=0)
    return scores.astype(np.float32)


# revision 18
# speedup vs baseline: 1.7720x; 1.7720x over previous
"""GNN message-passing (NBFNet-style) Trainium2 kernel: host prep + Bass/Tile builder.

v2.1 design (per core, 2 batches packed as 128 = 2b x 64d partitions):
  - node-state kept both in SBUF [128, NPAD] bf16 (rank-s space) and as HBM
    rank-major tables [NPAD, 128] bf16 (rebuilt per layer via DmaTranspose)
  - edge x-gather via GPSIMD dma_gather (transpose mode) from the HBM table
  - per-edge relation embeddings precomputed on host, streamed via plain DMA
  - segment stats via degree-sorted rounds (full-width, no halves) as in-place
    tensor_tensor prefix updates; the sq-sum chain and the sum-chain tail run
    on the GPSIMD/Pool engine (add-only TTs), with library switches between
    gather dispatch (mlp lib) and TT phases (standard lib)
  - PNA combine: 3 PSUM banks (one per deg-scale k); 13 matmuls + identity
    matmul folds the scale-multiplied k=1/k=2 terms into bank0; ACT relu evac.
    This removes the 8 per-tile "prod" tensors of the baseline.
  - stats post-processing split into 2 column blocks, matmul tiles run
    high-to-low, so matmuls overlap the (low-rank-column) tail of the rounds
"""
import sys
import contextlib

import numpy as np

sys.path.insert(0, "/opt/trn_rl_repo")
import ml_dtypes  # noqa: E402
import concourse.bass as bass  # noqa: E402
import concourse.tile as tile  # noqa: E402
from concourse import bacc, mybir, bass_utils, library_config  # noqa: E402

BF16 = mybir.dt.bfloat16
F32 = mybir.dt.float32
I16 = mybir.dt.int16
AF = mybir.ActivationFunctionType
OP = mybir.AluOpType
nbf = ml_dtypes.bfloat16

N = 5000
NPAD = 5120
E = 30000
D = 64
B = 16
NCORES = 8
BL = B // NCORES  # 2
HIST = 2
NL = 2
NEG = 32
NREL2 = 400
EPS = 1e-6
CH = 5120        # edge msg chunk (cols)
NTILE = 512      # matmul node tile
BLK = 2560       # stats post-processing block
HNP = NPAD // 2
EPS_CLIP_SCALE = 1e-2
SQ_POOL_ROUND = 0    # sq-chain rounds >= this run on Pool engine
SUM_POOL_ROUND = 8   # sum-chain rounds >= this run on Pool engine
EPAD = (E + 127) // 128 * 128  # gather-padded edge count (dma_gather %128)


def _wrap16(idx):
    """[L] int -> [128, L/16] int16 wrapped in 16 partitions, replicated x8."""
    L = len(idx)
    assert L % 16 == 0
    w = np.asarray(idx, np.int64).reshape(L // 16, 16).T.astype(np.int16)
    return np.tile(w, (8, 1))


def prep_snap(src, dst, et):
    """Host index preprocessing for one snapshot."""
    src = np.asarray(src, np.int64)
    dst = np.asarray(dst, np.int64)
    et = np.asarray(et, np.int64)
    indeg = np.bincount(dst, minlength=N).astype(np.int64)
    order = np.argsort(-indeg, kind="stable")  # rank -> node
    rank_of = np.empty(N, np.int64)
    rank_of[order] = np.arange(N)

    er = rank_of[dst]
    eord = np.argsort(er, kind="stable")  # edges sorted by dst rank
    er_s = er[eord]
    starts = np.searchsorted(er_s, er_s, side="left")
    slot = np.arange(E) - starts  # slot within dst group (0-indexed round)
    Rmax = int(indeg.max())
    W = np.array([int(np.count_nonzero(indeg > r)) for r in range(Rmax)], np.int64)
    off = np.concatenate([[0], np.cumsum(W)])
    assert off[-1] == E
    pos = off[slot] + er_s
    assert len(np.unique(pos)) == E
    src_rm = np.zeros(E, np.int64)
    et_rm = np.zeros(E, np.int64)
    src_rm[pos] = src[eord]
    et_rm[pos] = et[eord]

    # chunk op lists: per chunk (c0, c1, ops), ops = (msg_off, acc_off, width, round)
    chunks = []
    for c0 in range(0, E, CH):
        c1 = min(c0 + CH, E)
        ops = []
        for r in range(Rmax):
            g0, g1 = int(off[r]), int(off[r] + W[r])
            a, b_ = max(g0, c0), min(g1, c1)
            if a < b_:
                ops.append((a - c0, a - g0, b_ - a, r))
        chunks.append((c0, c1, ops))

    deg = (indeg + 1).astype(np.float64)
    scl = np.log(deg)
    scl = scl / scl.mean()
    iscl = 1.0 / np.clip(scl, EPS_CLIP_SCALE, None)
    invdeg = 1.0 / deg
    mask = (deg > 1).astype(np.float64)

    def pad_rank(x, fill):
        out = np.full(NPAD, fill, np.float64)
        out[:N] = x[order]
        return out

    v = np.stack([pad_rank(invdeg, 1.0), pad_rank(scl, 1.0), pad_rank(iscl, 1.0),
                  pad_rank(mask, 1.0)])
    return dict(
        indeg=indeg, order=order, rank_of=rank_of, W0=int(W[0]),
        src_rm=src_rm, et_rm=et_rm, chunks=chunks, v=v.astype(nbf),
    )


def preprocess(inputs):
    qt = np.asarray(inputs["query_triple"], np.int64)  # [B, NEG, 3]
    h_index, r_index, t_index = qt[..., 0], qt[..., 1], qt[..., 2]
    is_t_neg = np.all(h_index == h_index[:, :1], axis=-1, keepdims=True)
    h_i = np.where(is_t_neg, h_index, t_index)
    t_i = np.where(is_t_neg, t_index, h_index)
    r_i = np.where(is_t_neg, r_index, r_index + NREL2 // 2)

    ei = np.asarray(inputs["edge_index"], np.int64)
    etp = np.asarray(inputs["edge_type"], np.int64)
    snaps = [prep_snap(ei[s, 0], ei[s, 1], etp[s]) for s in range(HIST)]
    for s in range(HIST):
        sn = snaps[s]
        xi = np.zeros(EPAD, np.int64)
        xi[:E] = sn["rank_of"][sn["src_rm"]]
        sn["xidx_w"] = _wrap16(xi)
    # snap1 init gather: rank1 row j <- rank0 row r0[order1[j]]
    perm = np.zeros(NPAD, np.int64)
    perm[:N] = snaps[0]["rank_of"][snaps[1]["order"]]
    perm1_w = _wrap16(perm)

    qw = np.asarray(inputs["query_weight"], np.float32)      # [NREL2, D]
    rel = np.asarray(inputs["rel_embs"], np.float32)         # [NL, NREL2, D]
    lw = np.asarray(inputs["layer_Ws"], np.float32)          # [NL, 13*D, D]
    lb = np.asarray(inputs["layer_bs"], np.float32)          # [NL, D]
    w1 = np.asarray(inputs["mlp_w1"], np.float32)            # [128, 128]
    b1 = np.asarray(inputs["mlp_b1"], np.float32)            # [128]
    w2 = np.asarray(inputs["mlp_w2"], np.float32)            # [128, 1]
    b2 = np.asarray(inputs["mlp_b2"], np.float32)            # [1]

    # weight chunk tables: chunk 0 = rows 0:64 (x); chunk 1+si*3+k = rows
    # 64 + d*12 + si*3 + k (agg layout: ((d*4+si)*3+k))
    Wc = np.zeros((NL, 13, D, D), np.float32)
    for li in range(NL):
        Wc[li, 0] = lw[li, :D]
        for si in range(4):
            for k in range(3):
                rows = 64 + np.arange(D) * 12 + si * 3 + k
                Wc[li, 1 + si * 3 + k] = lw[li, rows]

    # per-edge rel tables [HIST*NL, 128, E] bf16 (rows = both batches x 64d)
    relw = np.zeros((HIST * NL, 128, E), nbf)
    for s in range(HIST):
        ets = snaps[s]["et_rm"]
        for li in range(NL):
            t = rel[li][ets].T.astype(nbf)   # [D, E]
            relw[s * NL + li, :D] = t
            relw[s * NL + li, D:] = t

    ident = np.eye(128, dtype=np.float32).astype(nbf)

    per_core = []
    for c in range(NCORES):
        bsl = slice(c * BL, (c + 1) * BL)
        q = qw[r_i[bsl, 0]]                      # [BL, D] f32
        init0 = np.zeros((NPAD, 128), np.float32)   # rank0-row-major
        for b in range(BL):
            init0[snaps[0]["rank_of"][h_i[c * BL + b, 0]], b * D:(b + 1) * D] = q[b]
        tid = np.zeros(128, np.int64)
        tid[:BL * NEG] = snaps[1]["rank_of"][t_i[bsl].reshape(-1)]
        per_core.append(dict(
            init0=init0.astype(nbf), qvec=q.astype(np.float32),
            tidx_w=_wrap16(tid),
        ))

    return dict(
        snaps=snaps, Wc=Wc, relw=relw, lbias=lb, perm1_w=perm1_w, ident=ident,
        w1=w1, b1=b1, w2=w2, b2=b2,
        per_core=per_core, h_i=h_i, t_i=t_i, r_i=r_i,
    )


def nm_ap(dram, r0=0, r1=NPAD):
    """HBM node-table rows [r0:r1) as [128, (r1-r0)/128, 128] tile AP."""
    return bass.AP(tensor=dram, offset=r0 * 128,
                   ap=[[128, 128], [128 * 128, (r1 - r0) // 128], [1, 128]])


def build(cfg, debug=()):
    nc = bacc.Bacc("TRN2", target_bir_lowering=False, debug=False,
                   dynamic_dma_scratch_size=16384)
    snaps = cfg["snaps"]

    # ---- DRAM tensors
    d_init0 = nc.dram_tensor("init0", [NPAD, 128], BF16, kind="ExternalInput")
    d_qvec = nc.dram_tensor("qvec", [BL, D], F32, kind="ExternalInput")
    d_tidx = nc.dram_tensor("tidx", [128, 8], I16, kind="ExternalInput")
    d_xidx = nc.dram_tensor("xidx", [HIST, 128, EPAD // 16], I16, kind="ExternalInput")
    d_perm1 = nc.dram_tensor("perm1", [128, NPAD // 16], I16, kind="ExternalInput")
    d_relw = nc.dram_tensor("relw", [HIST * NL, 128, E], BF16, kind="ExternalInput")
    d_v = nc.dram_tensor("vvec", [HIST, 4, NPAD], BF16, kind="ExternalInput")
    d_wc = nc.dram_tensor("wchunks", [NL, 13, D, D], F32, kind="ExternalInput")
    d_lb = nc.dram_tensor("lbias", [NL, D], F32, kind="ExternalInput")
    d_ident = nc.dram_tensor("ident", [128, 128], BF16, kind="ExternalInput")
    d_w1 = nc.dram_tensor("w1", [128, 128], F32, kind="ExternalInput")
    d_b1 = nc.dram_tensor("b1", [128], F32, kind="ExternalInput")
    d_w2 = nc.dram_tensor("w2", [128, 1], F32, kind="ExternalInput")
    d_b2 = nc.dram_tensor("b2", [1], F32, kind="ExternalInput")
    d_scores = nc.dram_tensor("scores", [BL, NEG], F32, kind="ExternalOutput")

    # rank-major work tables (rank space of their snapshot)
    d_tabs = {nm: nc.dram_tensor(nm, [NPAD, 128], BF16, kind="Internal")
              for nm in ("tabA", "tabB", "tabC", "tabD", "init1")}

    dbg_tensors = {}

    def dbg(name, shape, dtype):
        if name in debug:
            dbg_tensors[name] = nc.dram_tensor("dbg_" + name, shape, dtype,
                                               kind="ExternalOutput")
            return dbg_tensors[name]
        return None

    with tile.TileContext(nc) as tc, contextlib.ExitStack() as ctx:
        p_idx = ctx.enter_context(tc.tile_pool(name="idx", bufs=1))
        p_const = ctx.enter_context(tc.tile_pool(name="const", bufs=1))
        p_v = ctx.enter_context(tc.tile_pool(name="vrep", bufs=1))
        p_init = ctx.enter_context(tc.tile_pool(name="init", bufs=2))
        p_edge = ctx.enter_context(tc.tile_pool(name="edge", bufs=2))
        p_rel = ctx.enter_context(tc.tile_pool(name="rel", bufs=2))
        p_acc = ctx.enter_context(tc.tile_pool(name="acc", bufs=1))
        p_x = ctx.enter_context(tc.tile_pool(name="x", bufs=2))
        p_t12 = ctx.enter_context(tc.tile_pool(name="t12", bufs=4))
        p_ps = ctx.enter_context(tc.tile_pool(name="ps", bufs=6, space="PSUM"))
        p_ps2 = ctx.enter_context(tc.tile_pool(name="ps2", bufs=1, space="PSUM"))
        p_misc = ctx.enter_context(tc.tile_pool(name="misc", bufs=1))

        cur_lib = [None]

        def use_lib(lib):
            if cur_lib[0] is not lib:
                nc.gpsimd.load_library(lib)
                cur_lib[0] = lib

        # ---- setup: weights
        wmm = {}
        for li in range(NL):
            for c in range(13):
                wstg = p_misc.tile([D, D], F32, name="wstg", tag="wstg", bufs=2)
                nc.sync.dma_start(wstg[:], bass.AP(
                    tensor=d_wc, offset=(li * 13 + c) * D * D,
                    ap=[[D, D], [1, D]]))
                t = p_const.tile([128, 128], BF16, name=f"wmm{li}_{c}",
                                 tag=f"wmm{li}_{c}")
                nc.vector.memset(t[:], 0.0)
                nc.vector.tensor_copy(t[0:D, 0:D], wstg[:])
                nc.vector.tensor_copy(t[D:128, D:128], wstg[:])
                wmm[(li, c)] = t
        identsb = p_const.tile([128, 128], BF16, name="identsb", tag="identsb")
        nc.sync.dma_start(identsb[:], d_ident.ap()[:])
        w1stage = p_misc.tile([128, 128], F32, name="w1stage", tag="w1stage")
        nc.sync.dma_start(w1stage[:], d_w1.ap()[:])
        w1sb = p_const.tile([128, 128], BF16, name="w1sb", tag="w1sb")
        nc.vector.tensor_copy(w1sb[:], w1stage[:])
        w2stage = p_misc.tile([128, 1], F32, name="w2stage", tag="w2stage")
        nc.sync.dma_start(w2stage[:], d_w2.ap()[:])
        w2sb = p_const.tile([128, 1], BF16, name="w2sb", tag="w2sb")
        nc.vector.tensor_copy(w2sb[:], w2stage[:])
        b1sb = p_const.tile([128, 1], F32, name="b1sb", tag="b1sb")
        nc.sync.dma_start(b1sb[:], bass.AP(tensor=d_b1, offset=0, ap=[[1, 128], [1, 1]]))
        b2sb = p_const.tile([1, 1], F32, name="b2sb", tag="b2sb")
        nc.sync.dma_start(b2sb[:], bass.AP(tensor=d_b2, offset=0, ap=[[1, 1], [1, 1]]))
        lbsb = p_const.tile([128, NL], F32, name="lbsb", tag="lbsb")
        for li in range(NL):
            nc.sync.dma_start(lbsb[:, li:li + 1], bass.AP(
                tensor=d_lb, offset=li * D, ap=[[0, 2], [1, D]]))
        qsb = []
        for b in range(BL):
            t = p_const.tile([128, 1], F32, name=f"qsb{b}", tag=f"qsb{b}")
            nc.sync.dma_start(t[D:128, :], bass.AP(
                tensor=d_qvec, offset=b * D, ap=[[1, D], [1, 1]]))
            qsb.append(t)
        epssb = p_const.tile([128, 1], F32, name="epssb", tag="epssb")
        nc.vector.memset(epssb[:], EPS)
        tidx_sb = p_const.tile([128, 8], I16, name="tidx", tag="tidx")
        nc.sync.dma_start(tidx_sb[:], d_tidx.ap()[:])

        x_prev = None
        for s in range(HIST):
            sn = snaps[s]
            # ---- per-snap index arrays + v tensors
            xidx_sb = p_idx.tile([128, EPAD // 16], I16, name="xidx", tag="xidx")
            nc.sync.dma_start(xidx_sb[:], d_xidx.ap()[s])
            vrep = []
            for j in range(4):
                t = p_v.tile([128, NPAD], BF16, name=f"v{j}", tag=f"v{j}")
                nc.sync.dma_start(t[:], bass.AP(
                    tensor=d_v, offset=(s * 4 + j) * NPAD, ap=[[0, 128], [1, NPAD]]))
                vrep.append(t)
            vinv, vs1, vs2, vmask = vrep

            # ---- initial (paired with its square) in rank-s space
            init2 = p_init.tile([128, 2, NPAD], BF16, name="init2", tag="init2")
            i2v = init2[:, 0, :].rearrange("p (a b) -> p a b", b=128)
            if s == 0:
                in_tab = d_init0
                nc.sync.dma_start_transpose(i2v, d_init0.ap()[:])
            else:
                in_tab = d_tabs["init1"]
                perm_sb = p_idx.tile([128, NPAD // 16], I16, name="perm", tag="perm")
                nc.sync.dma_start(perm_sb[:], d_perm1.ap()[:])
                use_lib(library_config.mlp)
                a = p_rel.tile([128, CH], BF16, name="blenda", tag="relg")
                av = a[:].rearrange("p (a b) -> p a b", b=128)
                nc.gpsimd.dma_gather(
                    out_ap=av, in_ap=d_init0.ap()[:], idxs_ap=perm_sb[:],
                    num_idxs=NPAD, num_idxs_reg=NPAD, elem_size=128,
                    transpose=False, single_packet=False)
                bt = p_rel.tile([128, CH], BF16, name="blendb", tag="relg")
                bv = bt[:].rearrange("p (a b) -> p a b", b=128)
                nc.gpsimd.dma_gather(
                    out_ap=bv, in_ap=d_tabs["tabB"].ap()[:], idxs_ap=perm_sb[:],
                    num_idxs=NPAD, num_idxs_reg=NPAD, elem_size=128,
                    transpose=False, single_packet=False)
                nc.vector.tensor_tensor(out=a[:], in0=a[:], in1=bt[:], op=OP.add)
                nc.vector.tensor_scalar(out=a[:], in0=a[:], scalar1=0.5,
                                        scalar2=None, op0=OP.mult)
                nc.sync.dma_start(nm_ap(d_tabs["init1"]), av)
                nc.sync.dma_start_transpose(i2v, a[:])
            nc.scalar.activation(init2[:, 1, :], init2[:, 0, :], AF.Square)
            if (t_ := dbg(f"initial{s}", [128, 2, NPAD], BF16)) is not None:
                nc.sync.dma_start(t_.ap()[:], init2[:])

            tabs = [[in_tab, d_tabs["tabA" if s == 0 else "tabC"]],
                    [d_tabs["tabA" if s == 0 else "tabC"],
                     d_tabs["tabB" if s == 0 else "tabD"]]]
            for li in range(NL):
                xtab_in, xtab_out = tabs[li]
                x_in = init2[:, 0, :] if li == 0 else x_prev[:]
                W0 = sn["W0"]
                # ---- stats accumulators
                acc2 = p_acc.tile([128, 2, NPAD], BF16, name="acc2", tag="acc2")
                accmax = p_acc.tile([128, NPAD], BF16, name="accmax", tag="accmax")
                accmin = p_acc.tile([128, NPAD], BF16, name="accmin", tag="accmin")
                # suffix init (nodes with indeg==0 never touched by rounds)
                if W0 < NPAD:
                    nc.scalar.copy(acc2[:, :, W0:], init2[:, :, W0:])
                    nc.scalar.copy(accmax[:, W0:], init2[:, 0, W0:])
                    nc.scalar.copy(accmin[:, W0:], init2[:, 0, W0:])

                nch = len(sn["chunks"])

                def emit_gather(ci):
                    c0, c1, _ = sn["chunks"][ci]
                    g1 = (c1 - c0 + 127) // 128 * 128
                    xg = p_edge.tile([128, 1, CH], BF16, name="xg", tag="xg")
                    use_lib(library_config.mlp)
                    nc.gpsimd.dma_gather(
                        out_ap=xg[:, :, :g1], in_ap=xtab_in.ap()[:],
                        idxs_ap=xidx_sb[:, c0 // 16:(c0 + g1) // 16],
                        num_idxs=g1, num_idxs_reg=g1, elem_size=128,
                        transpose=True, single_packet=False)
                    return xg

                xgs = {0: emit_gather(0)}
                for ci, (c0, c1, ops) in enumerate(sn["chunks"]):
                    w = c1 - c0
                    xg = xgs.pop(ci)
                    relg = p_rel.tile([128, CH], BF16, name="relg", tag="relg")
                    nc.sync.dma_start(relg[:, :w],
                                      d_relw.ap()[s * NL + li][:, c0:c1])
                    msg = xg[:, 0, :w]
                    nc.vector.tensor_tensor(out=msg, in0=msg, in1=relg[:, :w],
                                            op=OP.mult)
                    # msgsq overwrites the (dead) rel values in-place
                    nc.scalar.activation(relg[:, :w], msg, AF.Square)
                    if ci == 0 and (t_ := dbg(f"xg{s}{li}", [128, CH], BF16)) is not None:
                        nc.sync.dma_start(t_.ap()[:, :w], msg)
                    if ci + 1 < nch:
                        xgs[ci + 1] = emit_gather(ci + 1)
                    pool_tt = []
                    for (mo, ao, wd, r) in ops:
                        # sum: tail rounds offloaded to Pool
                        in0 = init2[:, 0, ao:ao + wd] if r == 0 else acc2[:, 0, ao:ao + wd]
                        args = dict(out=acc2[:, 0, ao:ao + wd], in0=in0,
                                    in1=xg[:, 0, mo:mo + wd], op=OP.add)
                        if r >= SUM_POOL_ROUND:
                            pool_tt.append(args)
                        else:
                            nc.vector.tensor_tensor(**args)
                        # sq-sum: offloaded to Pool
                        in0 = init2[:, 1, ao:ao + wd] if r == 0 else acc2[:, 1, ao:ao + wd]
                        args = dict(out=acc2[:, 1, ao:ao + wd], in0=in0,
                                    in1=relg[:, mo:mo + wd], op=OP.add)
                        if r >= SQ_POOL_ROUND:
                            pool_tt.append(args)
                        else:
                            nc.vector.tensor_tensor(**args)
                        # max
                        in0 = init2[:, 0, ao:ao + wd] if r == 0 else accmax[:, ao:ao + wd]
                        nc.vector.tensor_tensor(
                            out=accmax[:, ao:ao + wd], in0=in0,
                            in1=xg[:, 0, mo:mo + wd], op=OP.max)
                        # min
                        in0 = init2[:, 0, ao:ao + wd] if r == 0 else accmin[:, ao:ao + wd]
                        nc.vector.tensor_tensor(
                            out=accmin[:, ao:ao + wd], in0=in0,
                            in1=xg[:, 0, mo:mo + wd], op=OP.min)
                    if pool_tt:
                        use_lib(library_config.standard)
                        for args in pool_tt:
                            nc.gpsimd.tensor_tensor(**args)

                # ---- stats post, per block (high block first to unlock matmuls)
                for b in range(NPAD // BLK - 1, -1, -1):
                    bsl = slice(b * BLK, (b + 1) * BLK)
                    nc.vector.tensor_tensor(
                        out=acc2[:, :, bsl], in0=acc2[:, :, bsl],
                        in1=vinv[:, bsl].unsqueeze(1).broadcast_to([128, 2, BLK]),
                        op=OP.mult)
                    msq = p_rel.tile([128, CH], BF16, name="msq", tag="relg")
                    msq = msq[:, :BLK]
                    nc.scalar.activation(msq, acc2[:, 0, bsl], AF.Square)
                    nc.vector.tensor_tensor(out=acc2[:, 1, bsl],
                                            in0=acc2[:, 1, bsl], in1=msq,
                                            op=OP.subtract)
                    nc.scalar.activation(acc2[:, 1, bsl], acc2[:, 1, bsl], AF.Relu)
                    nc.vector.tensor_tensor(out=acc2[:, 1, bsl],
                                            in0=acc2[:, 1, bsl],
                                            in1=vmask[:, bsl], op=OP.mult)
                    nc.scalar.activation(acc2[:, 1, bsl], acc2[:, 1, bsl],
                                         AF.Sqrt, bias=epssb[:, 0:1])
                if (t_ := dbg(f"stats{s}{li}", [128, 2, NPAD], BF16)) is not None:
                    nc.sync.dma_start(t_.ap()[:], acc2[:])

                # ---- matmuls: high tiles first; writeback per half
                xnext = p_x.tile([128, NPAD], BF16, name="xnext", tag="xnext")
                stat_rhs = [acc2[:, 0, :], accmax[:], accmin[:], acc2[:, 1, :]]
                for t in range(NPAD // NTILE - 1, -1, -1):
                    tsl = slice(t * NTILE, (t + 1) * NTILE)
                    ps = [p_ps.tile([128, NTILE], F32, name=f"ps{k}", tag="ps")
                          for k in range(3)]
                    for k in (1, 2):
                        for si in range(4):
                            nc.tensor.matmul(
                                out=ps[k][:], lhsT=wmm[(li, 1 + si * 3 + k)][:],
                                rhs=stat_rhs[si][:, tsl],
                                start=(si == 0), stop=(si == 3))
                    nc.tensor.matmul(out=ps[0][:], lhsT=wmm[(li, 0)][:],
                                     rhs=x_in[:, tsl], start=True, stop=False)
                    for si in range(4):
                        nc.tensor.matmul(
                            out=ps[0][:], lhsT=wmm[(li, 1 + si * 3)][:],
                            rhs=stat_rhs[si][:, tsl], start=False, stop=False)
                    t1 = p_t12.tile([128, NTILE], BF16, name="t1", tag="t1")
                    t2 = p_t12.tile([128, NTILE], BF16, name="t2", tag="t2")
                    nc.scalar.copy(t1[:], ps[1][:])
                    nc.scalar.copy(t2[:], ps[2][:])
                    nc.vector.tensor_tensor(out=t1[:], in0=t1[:],
                                            in1=vs1[:, tsl], op=OP.mult)
                    nc.vector.tensor_tensor(out=t2[:], in0=t2[:],
                                            in1=vs2[:, tsl], op=OP.mult)
                    nc.vector.tensor_tensor(out=t1[:], in0=t1[:], in1=t2[:],
                                            op=OP.add)
                    nc.tensor.matmul(out=ps[0][:], lhsT=identsb[:], rhs=t1[:],
                                     start=False, stop=True)
                    nc.scalar.activation(xnext[:, tsl], ps[0][:], AF.Relu,
                                         bias=lbsb[:, li:li + 1])
                    # per-half transposed writeback as soon as a half is done
                    if t in (NPAD // NTILE // 2, 0):
                        h0 = t * NTILE
                        tout = p_rel.tile([128, CH], BF16, name="tout", tag="relg")
                        tv = tout[:, :HNP].rearrange("p (a b) -> p a b", b=128)
                        nc.sync.dma_start_transpose(tv, xnext[:, h0:h0 + HNP])
                        nc.sync.dma_start(nm_ap(xtab_out, h0, h0 + HNP), tv)
                if (t_ := dbg(f"x_s{s}_l{li}", [128, NPAD], BF16)) is not None:
                    nc.sync.dma_start(t_.ap()[:], xnext[:])
                x_prev = xnext

        # ---- final readout from tabD (rank-1 space)
        use_lib(library_config.mlp)
        tg = p_misc.tile([128, 1, 128], BF16, name="tg", tag="tg")
        nc.gpsimd.dma_gather(
            out_ap=tg[:], in_ap=d_tabs["tabD"].ap()[:], idxs_ap=tidx_sb[:],
            num_idxs=128, num_idxs_reg=128, elem_size=128, transpose=True,
            single_packet=False)
        tg = tg[:, 0, :]
        for b in range(BL):
            ft = p_misc.tile([128, NEG], BF16, name=f"ft{b}", tag=f"ft{b}")
            if b == 0:
                nc.vector.tensor_copy(ft[0:D, :], tg[0:D, 0:NEG])
            else:
                nc.sync.dma_start(ft[0:D, :], tg[D:128, b * NEG:(b + 1) * NEG])
            nc.vector.memset(ft[D:128, :], 0.0)
            nc.vector.tensor_scalar(out=ft[D:128, :], in0=ft[D:128, :],
                                    scalar1=qsb[b][D:128, 0:1], scalar2=None,
                                    op0=OP.add)
            ps1 = p_ps2.tile([128, NEG], F32, name="mlp1", tag="mlp1")
            nc.tensor.matmul(out=ps1[:], lhsT=w1sb[:], rhs=ft[:], start=True, stop=True)
            hd = p_misc.tile([128, NEG], BF16, name=f"hd{b}", tag=f"hd{b}")
            nc.scalar.activation(hd[:], ps1[:], AF.Relu, bias=b1sb[:, 0:1])
            ps2 = p_ps2.tile([1, NEG], F32, name="mlp2", tag="mlp2")
            nc.tensor.matmul(out=ps2[:], lhsT=w2sb[:], rhs=hd[:], start=True, stop=True)
            sc = p_misc.tile([1, NEG], F32, name=f"sc{b}", tag=f"sc{b}")
            nc.vector.tensor_scalar(out=sc[:], in0=ps2[:], scalar1=b2sb[0:1, 0:1],
                                    scalar2=None, op0=OP.add)
            nc.sync.dma_start(d_scores.ap()[b:b + 1, :], sc[:])

    nc.compile()
    return nc, dbg_tensors


def make_in_maps(cfg):
    shared = {}
    shared["xidx"] = np.stack([cfg["snaps"][s]["xidx_w"] for s in range(HIST)])
    shared["perm1"] = cfg["perm1_w"]
    shared["relw"] = cfg["relw"]
    shared["vvec"] = np.stack([cfg["snaps"][s]["v"] for s in range(HIST)])
    shared["wchunks"] = cfg["Wc"]
    shared["lbias"] = cfg["lbias"]
    shared["ident"] = cfg["ident"]
    shared["w1"] = cfg["w1"]
    shared["b1"] = cfg["b1"]
    shared["w2"] = cfg["w2"]
    shared["b2"] = cfg["b2"]
    in_maps = []
    for c in range(NCORES):
        pc = cfg["per_core"][c]
        in_maps.append(dict(shared, init0=pc["init0"], qvec=pc["qvec"],
                            tidx=pc["tidx_w"]))
    return in_maps


_CACHE = {}


def _get_program(cfg, key):
    if key not in _CACHE:
        _CACHE[key] = build(cfg)
    return _CACHE[key]


def kernel(**inputs):
    cfg = preprocess(inputs)
    key = (np.asarray(inputs["edge_index"]).tobytes(),
           np.asarray(inputs["edge_type"]).tobytes())
    nc, _ = _get_program(cfg, key)
    in_maps = make_in_maps(cfg)
    res = bass_utils.run_bass_kernel_spmd(nc, in_maps, core_ids=list(range(NCORES)))
    scores = np.concatenate([res.results[c]["scores"] for c in range(NCORES)], axis=0)
    return scores.astype(np.float32)
